# revision 9
# baseline (speedup 1.0000x reference)
"""Causal full attention (B=4, L=S=2048, H=8, E=D=64) on 8 Trainium2 NeuronCores.

Strategy (per core, 4 (b,h) heads; B*H=32 pairs sharded 4-per-core):
  - Host pre-transposes Q,K to [E,L] layout (bf16), appends a ones-column to V
    (for softmax denominators), and folds attn_mask + additive causal_mask bias
    into a single multiplicative table E_bias^T[s,l] = exp(scale*bias) (0 where
    masked), so no max-subtraction or separate mask op is needed on device.
  - Device computes transposed score blocks S^T[s,l] = K^T.T @ Q^T on the PE,
    exp() on the scalar engine (PSUM -> SBUF bf16), multiplies by E_bias^T on
    the vector engine (bf16 2x mode), and accumulates the output in natural
    [l, d] layout with lhsT = P^T block slices, rhs = V_aug chunks.  Column 64
    of the accumulator is the softmax denominator; a per-partition reciprocal +
    tensor_scalar multiply normalizes on eviction.
"""

import sys

for _p in ("/opt/trn_rl_repo",):
    if _p not in sys.path:
        sys.path.insert(0, _p)

import numpy as np
import ml_dtypes

B, L, S, H, E, D = 4, 2048, 2048, 8, 64, 64
SCALE = 1.0 / 8.0  # 1/sqrt(E)
N_CORES = 8
HPC = 4            # heads (b,h flat) per core
NRANGE = 4         # l ranges of 512
RW = 512           # l range width
NCHUNK = 16        # s chunks of 128
P = 128

_compiled = {}     # (causal,) -> Bass module
_MASKED = -1e30


def _build(causal: bool):
    import concourse.bass as bass
    import concourse.tile as tile
    from concourse import bacc, mybir
    from concourse.bass import broadcast_tensor_aps

    bf16 = mybir.dt.bfloat16
    f32 = mybir.dt.float32
    Exp = mybir.ActivationFunctionType.Exp

    nc = bacc.Bacc("TRN2", target_bir_lowering=False, debug=False,
                   num_devices=N_CORES)

    # q/k stored as head pairs: [pair, 128, L] with rows 0:64 = head 2p,
    # rows 64:128 = head 2p+1 (enables row-tiled concurrent matmuls)
    qt_d = nc.dram_tensor("qt", [HPC // 2, 2 * E, L], bf16,
                          kind="ExternalInput").ap()
    kt_d = nc.dram_tensor("kt", [HPC // 2, 2 * E, S], bf16,
                          kind="ExternalInput").ap()
    va_d = nc.dram_tensor("va", [HPC, P, NCHUNK, D + 1], bf16,
                          kind="ExternalInput").ap()
    eb_d = nc.dram_tensor("eb", [S, L], bf16, kind="ExternalInput").ap()
    out_d = nc.dram_tensor("out", [HPC, L, D], f32, kind="ExternalOutput").ap()

    def jmax(r):
        # last s-chunk participating in l-range r
        return 4 * r + 3 if causal else NCHUNK - 1

    with tile.TileContext(nc) as tc:
        with (
            tc.tile_pool(name="const", bufs=1) as const,
            tc.tile_pool(name="ebp", bufs=8) as ebp,
            tc.tile_pool(name="pp", bufs=6) as pp,
            tc.tile_pool(name="scp", bufs=1, space="PSUM") as scp,
            tc.tile_pool(name="avp", bufs=1, space="PSUM") as avp,
            tc.tile_pool(name="outp", bufs=4) as outp,
        ):
            qt_sb, kt_sb, va_sb = [], [], []
            for pr in range(HPC // 2):
                q_t = const.tile([2 * E, L], bf16, name=f"qt{pr}")
                qt_sb.append(q_t)
                k_t = const.tile([2 * E, S], bf16, name=f"kt{pr}")
                kt_sb.append(k_t)
            for h in range(HPC):
                v_t = const.tile([P, NCHUNK, D + 1], bf16, name=f"va{h}")
                va_sb.append(v_t)
            def load_chunk(c, eng=None):
                # kt/qt/va columns first needed by l-range c; prefetches go
                # on the SWDGE (gpsimd) queue so they never convoy the sync
                # queue's eb/out stream (chunk 0 uses sync: lowest latency)
                eng = eng or nc.gpsimd
                cs = slice(RW * c, RW * c + RW)
                for pr in range(HPC // 2):
                    eng.dma_start(out=kt_sb[pr][:, cs], in_=kt_d[pr][:, cs])
                    eng.dma_start(out=qt_sb[pr][:, cs], in_=qt_d[pr][:, cs])
                for h in range(HPC):
                    eng.dma_start(out=va_sb[h][:, 4 * c:4 * c + 4, :],
                                  in_=va_d[h][:, 4 * c:4 * c + 4, :])

            load_chunk(0, eng=nc.sync)

            for r in range(NRANGE):
                av = [avp.tile([P, 4, D + 1], f32, tag=f"av{h}",
                               name=f"av{h}_{r}") for h in range(HPC)]

                def emit_av(work):
                    rr, j, hf, p_t = work
                    for hh in range(2):
                        h = 2 * hf + hh
                        for t in range(4):
                            tg = 4 * rr + t
                            if causal and j > tg:
                                continue
                            # start/stop granularity is the whole 2KB PSUM
                            # bank (zero region), so the four t-slices of
                            # av[h] form one accumulation group
                            nc.tensor.matmul(
                                av[h][:, t:t + 1, :],
                                lhsT=p_t[:, RW * hh + 128 * t:
                                         RW * hh + 128 * t + 128],
                                rhs=va_sb[h][:, j, :],
                                start=(j == 0 and t == 0),
                                stop=(j == jmax(rr) and t == 3))

                pending = None
                if causal and r < NRANGE - 1:
                    load_chunk(r + 1)
                elif not causal and r == 0:
                    for c in range(1, 4):
                        load_chunk(c)
                for j in range(jmax(r) + 1):
                    # causal trim: within a diagonal block only l >= s
                    # columns are live
                    off = max(0, 128 * (j - 4 * r)) if causal else 0
                    W = RW - off
                    ebt = ebp.tile([P, RW], bf16, name=f"eb_{r}_{j}", tag="eb")
                    nc.sync.dma_start(
                        out=ebt[:, :W],
                        in_=eb_d[128 * j:128 * j + 128,
                                 RW * r + off:RW * r + RW])
                    # two 2-head halves so ACT exp on one half overlaps PE
                    # scores on the other (each half = 2 PSUM banks)
                    for hf in range(2):
                        sc = scp.tile([P, 2 * RW], f32,
                                      name=f"sc{hf}_{r}_{j}", tag=f"sc{hf}")
                        for hh in range(2):
                            # row-tiled pair: head hh of pair hf lives on
                            # array rows/partitions 64*hh .. 64*hh+63
                            nc.tensor.matmul(
                                sc[:, RW * hh + off:RW * hh + RW],
                                lhsT=kt_sb[hf][64 * hh:64 * hh + 64,
                                               128 * j:128 * j + 128],
                                rhs=qt_sb[hf][64 * hh:64 * hh + 64,
                                              RW * r + off:RW * r + RW],
                                start=True, stop=True,
                                tile_position=(64 * hh, 0))
                        # AV of the previous half slots in behind these
                        # scores on the PE queue, hiding the exp+mult wait
                        if pending is not None:
                            emit_av(pending)
                        p_t = pp.tile([P, 2 * RW], bf16,
                                      name=f"p{hf}_{r}_{j}", tag=f"p{hf}")
                        sc3 = sc.rearrange("p (hh c) -> p hh c", hh=2)
                        p3 = p_t.rearrange("p (hh c) -> p hh c", hh=2)
                        nc.scalar.activation(p3[:, :, off:], sc3[:, :, off:],
                                             Exp, scale=SCALE)
                        # single DVE op for both heads: E_bias block
                        # broadcast along the head axis via a 0-step AP
                        p3s = p3[:, :, off:]
                        e3 = ebt[:, :W].rearrange("p (x c) -> p x c", x=1)
                        _, e3b = broadcast_tensor_aps(p3s, e3)
                        nc.vector.tensor_mul(p3s, p3s, e3b)
                        pending = (r, j, hf, p_t)
                if pending is not None:
                    emit_av(pending)
                    pending = None
                # normalize + store range r
                for h in range(HPC):
                    rec = outp.tile([P, 4, 1], f32, name=f"rec_{r}_{h}", tag="rec")
                    nc.vector.reciprocal(rec[:], av[h][:, :, D:D + 1])
                    o_t = outp.tile([P, 4, D], f32, name=f"o_{r}_{h}", tag="o")
                    avs = av[h][:, :, 0:D]
                    _, recb = broadcast_tensor_aps(avs, rec)
                    nc.vector.tensor_mul(o_t[:], avs, recb)
                    nc.sync.dma_start(
                        out=out_d[h, RW * r:RW * r + RW, :].rearrange(
                            "(t p) d -> p t d", p=P),
                        in_=o_t[:])
    nc.compile()
    return nc


def _get_nc(causal: bool):
    key = (causal,)
    if key not in _compiled:
        _compiled[key] = _build(causal)
    return _compiled[key]


def kernel(queries, keys, values, causal_mask, attn_mask):
    from concourse.bass_utils import run_bass_kernel_spmd

    bf = ml_dtypes.bfloat16
    mask2d = np.asarray(attn_mask).reshape(L, S)
    causal = bool(
        (mask2d == np.triu(np.ones((L, S), dtype=bool), k=1)).all())

    # E_bias^T[s, l] = exp(scale * bias[l, s]), 0 where masked
    bias = np.where(mask2d, -np.inf, np.asarray(causal_mask, np.float32))
    ebT = np.exp(SCALE * bias.T).astype(bf)

    # [B,L,H,E] -> [B,H,E,L] -> flat heads [32, E, L]
    qt = np.ascontiguousarray(
        np.asarray(queries, np.float32).transpose(0, 2, 3, 1)
    ).reshape(B * H, E, L).astype(bf)
    kt = np.ascontiguousarray(
        np.asarray(keys, np.float32).transpose(0, 2, 3, 1)
    ).reshape(B * H, E, S).astype(bf)

    # V + ones column, laid out [head, p, chunk, D+1] with s = 128*chunk + p
    v4 = np.asarray(values, np.float32).transpose(0, 2, 1, 3).reshape(
        B * H, NCHUNK, P, D)
    va = np.concatenate(
        [v4, np.ones((B * H, NCHUNK, P, 1), np.float32)], axis=-1)
    va = np.ascontiguousarray(va.transpose(0, 2, 1, 3)).astype(bf)

    nc = _get_nc(causal)
    in_maps = []
    for c in range(N_CORES):
        sl = slice(HPC * c, HPC * (c + 1))
        in_maps.append({
            "qt": np.ascontiguousarray(qt[sl]).reshape(HPC // 2, 2 * E, L),
            "kt": np.ascontiguousarray(kt[sl]).reshape(HPC // 2, 2 * E, S),
            "va": np.ascontiguousarray(va[sl]),
            "eb": ebT,
        })
    res = run_bass_kernel_spmd(nc, in_maps, core_ids=list(range(N_CORES)))

    out = np.empty((B, L, H, D), np.float32)
    for c in range(N_CORES):
        for hl in range(HPC):
            k = HPC * c + hl
            out[k // H, :, k % H, :] = res.results[c]["out"][hl]
    return out


if __name__ == "__main__":
    rng = np.random.default_rng(0)
    q = rng.standard_normal((B, L, H, E), dtype=np.float32)
    k = rng.standard_normal((B, S, H, E), dtype=np.float32)
    v = rng.standard_normal((B, S, H, D), dtype=np.float32)
    cm = rng.standard_normal((L, S), dtype=np.float32)
    am = np.triu(np.ones((L, S), dtype=bool), k=1)[None, None]
    o = kernel(queries=q, keys=k, values=v, causal_mask=cm, attn_mask=am)
    print(o.shape, o.dtype, np.abs(o).mean())
